# revision 1
# baseline (speedup 1.0000x reference)
"""MetaGRU (gnn_message_passing) Trainium2 kernel.

Strategy:
  - 320000 edges sharded 8 ways (40000/core, padded to 40960).
  - Device (per GRU step, one SPMD launch on 8 cores): the edge model +
    edge GRU — all matmuls vs. per-edge feature vectors, in bf16
    feature-major layout [128 features x E_loc edges] with fp32 PSUM
    accumulation.  This is ~85% of the problem's FLOPs.
  - Host (exact fp32 numpy): gather tables xa[src]+xb[dst] (the graph
    gather), scatter segment-sums, the tiny node GRU (10k rows) and
    global GRU (16 rows) between launches.
  - 3 launches reuse one Bass program (NEFF cache hit after the first).
"""
import os
import sys

sys.path.insert(0, "/opt/trn_rl_repo")

import numpy as np
from ml_dtypes import bfloat16

import concourse.bass as bass
import concourse.bacc as bacc_mod
import concourse.mybir as mybir
from concourse.tile import TileContext, add_dep_helper
from concourse.bass_utils import run_bass_kernel_spmd

H = 128
E = 320000
NCORES = 8
E_SHARD = E // NCORES          # 40000
TILE = 512
E_LOC = 40960                  # E_SHARD padded up to a multiple of TILE
NTILES = E_LOC // TILE

AF = mybir.ActivationFunctionType
OP = mybir.AluOpType
BF16 = mybir.dt.bfloat16
F32 = mybir.dt.float32


def build_nc(e_loc=E_LOC):
    """One edge-GRU step for one shard of e_loc edges.

    inputs (per core):
      g    [128, e_loc] bf16  : (x@We1 + u[batch]@We4)[src] + (x@We2)[dst], transposed
      ea   [128, e_loc] bf16  : edge_attr^T shard
      wts  [128, 1024]  bf16  : [We3 | I | ihr | ihz | ihn | hhr | hhz | hhn]
      bias [128, 5]     f32   : [be | br | bz | bhn | bin] as columns
    output:
      eao  [128, e_loc] bf16  : new edge_attr^T shard
    """
    ntiles = e_loc // TILE
    nc = bacc_mod.Bacc()
    g_d = nc.declare_dram_parameter("g", [H, e_loc], BF16, isOutput=False)
    ea_d = nc.declare_dram_parameter("ea", [H, e_loc], BF16, isOutput=False)
    w_d = nc.declare_dram_parameter("wts", [H, 8 * H], BF16, isOutput=False)
    b_d = nc.declare_dram_parameter("bias", [H, 5], F32, isOutput=False)
    br_d = nc.declare_dram_parameter("brow", [1, 5 * H], BF16, isOutput=False)
    o_d = nc.declare_dram_parameter("eao", [H, e_loc], BF16, isOutput=True)

    with TileContext(nc) as tc:
        with (
            tc.tile_pool(name="const", bufs=1) as cpool,
            tc.tile_pool(name="sb", bufs=2) as pool,
            tc.tile_pool(name="ps", bufs=8, space="PSUM") as pspool,
        ):
            w_sb = cpool.tile([H, 8 * H], BF16)
            nc.sync.dma_start(out=w_sb[:], in_=w_d[:])
            b_sb = cpool.tile([H, 5], F32)
            nc.sync.dma_start(out=b_sb[:], in_=b_d[:])
            br_sb = cpool.tile([1, 5 * H], BF16)
            nc.sync.dma_start(out=br_sb[:], in_=br_d[:])
            ones_sb = cpool.tile([1, TILE], BF16)
            nc.vector.memset(ones_sb[:], 1.0)

            g_sb = cpool.tile([H, e_loc], BF16)
            ea_sb = cpool.tile([H, e_loc], BF16)
            scratch = cpool.tile([H, 1], BF16)
            nchunk = 4
            csz = e_loc // nchunk
            for c in range(nchunk):
                cs = slice(c * csz, (c + 1) * csz)
                nc.sync.dma_start(out=g_sb[:, cs], in_=g_d[:, cs])
                nc.sync.dma_start(out=ea_sb[:, cs], in_=ea_d[:, cs])

            obs_in = nc.vector.tensor_copy(scratch[:], ea_sb[:, e_loc - 1:e_loc])

            def W(k):
                return w_sb[:, k * H:(k + 1) * H]

            # weight slots: 0 We3, 1 I, 2 ihr, 3 ihz, 4 ihn, 5 hhr, 6 hhz, 7 hhn
            def B(k):
                return b_sb[:, k:k + 1]

            for t in range(ntiles):
                sl = slice(t * TILE, (t + 1) * TILE)
                g_t = g_sb[:, sl]
                ea_t = ea_sb[:, sl]
                ea_t2 = ea_sb[:, sl]

                def S(x, j):
                    return x[:, j * 512:(j + 1) * 512]

                # pre = We3^T @ ea + g   (+be and relu on DVE below)
                pre = pspool.tile([H, TILE], F32, tag="ps")
                nc.tensor.matmul(pre[:], W(0), ea_t[:], start=True, stop=False)
                nc.tensor.matmul(pre[:], W(1), g_t[:], start=False, stop=False)
                nc.tensor.matmul(pre[:], br_sb[0:1, 0:H], ones_sb[:], start=False, stop=True)
                eo = pool.tile([H, TILE], BF16, tag="eo")
                # e_out = max(pre + be, 0)   (be came in via the bias-row matmul)
                obs_p = nc.vector.tensor_copy(scratch[:], pre[:, 0:1])
                i_eo = nc.vector.tensor_scalar_max(eo[:], pre[:], 0.0)
                add_dep_helper(i_eo.ins, obs_p.ins, False)
                if t == 0:
                    add_dep_helper(i_eo.ins, obs_in.ins, False)

                # r = sigmoid(ihr^T@eo + hhr^T@ea + br)
                rp = pspool.tile([H, TILE], F32, tag="ps")
                nc.tensor.matmul(rp[:], W(2), eo[:], start=True, stop=False)
                nc.tensor.matmul(rp[:], W(5), ea_t[:], start=False, stop=False)
                nc.tensor.matmul(rp[:], br_sb[0:1, 2 * H:3 * H], ones_sb[:], start=False, stop=True)
                r = pool.tile([H, TILE], BF16, tag="r")
                nc.scalar.activation(r[:], rp[:], AF.Sigmoid)

                # z = sigmoid(ihz^T@eo + hhz^T@ea + bz)
                zp = pspool.tile([H, TILE], F32, tag="ps")
                nc.tensor.matmul(zp[:], W(3), eo[:], start=True, stop=False)
                nc.tensor.matmul(zp[:], W(6), ea_t[:], start=False, stop=False)
                nc.tensor.matmul(zp[:], br_sb[0:1, 3 * H:4 * H], ones_sb[:], start=False, stop=True)
                z = pool.tile([H, TILE], BF16, tag="z")
                nc.scalar.activation(z[:], zp[:], AF.Sigmoid)
                obs_z = nc.vector.tensor_copy(scratch[:], z[:, 0:1])

                # m = r * (hhn^T@ea + bhn)
                hnp = pspool.tile([H, TILE], F32, tag="ps")
                nc.tensor.matmul(hnp[:], W(7), ea_t[:], start=True, stop=False)
                nc.tensor.matmul(hnp[:], br_sb[0:1, H:2 * H], ones_sb[:], start=False, stop=True)
                hnb = pool.tile([H, TILE], BF16, tag="hnb")
                obs_h = nc.vector.tensor_copy(scratch[:], hnp[:, 0:1])
                i_hnb = nc.vector.tensor_copy(hnb[:], hnp[:])
                add_dep_helper(i_hnb.ins, obs_h.ins, False)
                m = pool.tile([H, TILE], BF16, tag="m")
                i_m = nc.vector.tensor_mul(m[:], r[:], hnb[:])
                add_dep_helper(i_m.ins, obs_z.ins, False)

                # n = tanh(ihn^T@eo + m + bin)
                inp = pspool.tile([H, TILE], F32, tag="ps")
                nc.tensor.matmul(inp[:], W(4), eo[:], start=True, stop=False)
                nc.tensor.matmul(inp[:], W(1), m[:], start=False, stop=False)
                nc.tensor.matmul(inp[:], br_sb[0:1, 4 * H:5 * H], ones_sb[:], start=False, stop=True)
                n_t = pool.tile([H, TILE], BF16, tag="n")
                nc.scalar.activation(n_t[:], inp[:], AF.Tanh)
                obs_n = nc.vector.tensor_copy(scratch[:], n_t[:, 0:1])

                # h' = n + z*(h - n)
                d = pool.tile([H, TILE], BF16, tag="d")
                i_d = nc.vector.tensor_sub(d[:], ea_t2[:], n_t[:])
                add_dep_helper(i_d.ins, obs_n.ins, False)
                zd = pool.tile([H, TILE], BF16, tag="zd")
                nc.vector.tensor_mul(zd[:], z[:], d[:])
                h = pool.tile([H, TILE], BF16, tag="h")
                nc.vector.tensor_add(h[:], n_t[:], zd[:])
                nc.sync.dma_start(out=o_d[:, sl], in_=h[:])
    nc.compile()
    return nc


def _sigmoid(x):
    return 1.0 / (1.0 + np.exp(-x))


def _gru_np(inp, h, Wih, Whh, bih, bhh):
    gi = inp @ Wih + bih
    gh = h @ Whh + bhh
    i_r, i_z, i_n = np.split(gi, 3, axis=-1)
    h_r, h_z, h_n = np.split(gh, 3, axis=-1)
    r = _sigmoid(i_r + h_r)
    z = _sigmoid(i_z + h_z)
    n = np.tanh(i_n + r * h_n)
    return (1.0 - z) * n + z * h


_NC_CACHE = {}
_RUNNER_CACHE = {}
LAST_EXEC_NS = []  # per-launch wall-clock ns


def _get_runner(nc):
    """Jit the SPMD executable once; reuse across launches (jax cache)."""
    key = id(nc)
    if key in _RUNNER_CACHE:
        return _RUNNER_CACHE[key]
    import jax
    from jax.sharding import Mesh, PartitionSpec
    from jax.experimental.shard_map import shard_map
    import concourse.mybir as mb
    from concourse import bass2jax as b2j

    b2j.install_neuronx_cc_hook()
    partition_name = nc.partition_id_tensor.name if nc.partition_id_tensor else None
    in_names, out_names, out_avals, zero_outs = [], [], [], []
    for alloc in nc.m.functions[0].allocations:
        if not isinstance(alloc, mb.MemoryLocationSet):
            continue
        name = alloc.memorylocations[0].name
        if alloc.kind == "ExternalInput":
            if name != partition_name:
                in_names.append(name)
        elif alloc.kind == "ExternalOutput":
            shape = tuple(alloc.tensor_shape)
            dtype = mb.dt.np(alloc.dtype)
            out_avals.append(jax.core.ShapedArray(shape, dtype))
            out_names.append(name)
            zero_outs.append(np.zeros(shape, dtype))
    n_params = len(in_names)
    n_outs = len(out_avals)
    all_in_names = list(in_names) + list(out_names)
    if partition_name is not None:
        all_in_names.append(partition_name)
    donate = tuple(range(n_params, n_params + n_outs))

    def _body(*args):
        operands = list(args)
        if partition_name is not None:
            operands.append(b2j.partition_id_tensor())
        outs = b2j._bass_exec_p.bind(
            *operands,
            out_avals=tuple(out_avals),
            in_names=tuple(all_in_names),
            out_names=tuple(out_names),
            lowering_input_output_aliases=(),
            sim_require_finite=True,
            sim_require_nnan=True,
            nc=nc,
        )
        return tuple(outs)

    devices = jax.devices()[:NCORES]
    mesh = Mesh(np.asarray(devices), ("core",))
    in_specs = (PartitionSpec("core"),) * (n_params + n_outs)
    out_specs = (PartitionSpec("core"),) * n_outs
    sharded = jax.jit(
        shard_map(_body, mesh=mesh, in_specs=in_specs, out_specs=out_specs,
                  check_rep=False),
        donate_argnums=donate, keep_unused=True,
    )

    def run(in_maps):
        per_core = [[np.asarray(m[nm]) for nm in in_names] for m in in_maps]
        concat_in = [
            np.concatenate([per_core[c][i] for c in range(NCORES)], axis=0)
            for i in range(n_params)
        ]
        concat_zeros = [
            np.zeros((NCORES * z.shape[0], *z.shape[1:]), z.dtype) for z in zero_outs
        ]
        out_arrs = sharded(*concat_in, *concat_zeros)
        return [
            {nm: np.asarray(out_arrs[i]).reshape(NCORES, *out_avals[i].shape)[c]
             for i, nm in enumerate(out_names)}
            for c in range(NCORES)
        ]

    _RUNNER_CACHE[key] = run
    return run


def kernel(**inputs):
    x = np.asarray(inputs["x"], np.float32)
    ea = np.asarray(inputs["edge_attr"], np.float32)
    u = np.asarray(inputs["u"], np.float32)
    We = np.asarray(inputs["We"], np.float32)
    be = np.asarray(inputs["be"], np.float32)
    Wn = np.asarray(inputs["Wn"], np.float32)
    bn = np.asarray(inputs["bn"], np.float32)
    Wg = np.asarray(inputs["Wg"], np.float32)
    bg = np.asarray(inputs["bg"], np.float32)
    eWih = np.asarray(inputs["eWih"], np.float32)
    eWhh = np.asarray(inputs["eWhh"], np.float32)
    ebih = np.asarray(inputs["ebih"], np.float32)
    ebhh = np.asarray(inputs["ebhh"], np.float32)
    nWih = np.asarray(inputs["nWih"], np.float32)
    nWhh = np.asarray(inputs["nWhh"], np.float32)
    nbih = np.asarray(inputs["nbih"], np.float32)
    nbhh = np.asarray(inputs["nbhh"], np.float32)
    gWih = np.asarray(inputs["gWih"], np.float32)
    gWhh = np.asarray(inputs["gWhh"], np.float32)
    gbih = np.asarray(inputs["gbih"], np.float32)
    gbhh = np.asarray(inputs["gbhh"], np.float32)
    edge_index = np.asarray(inputs["edge_index"])
    batch = np.asarray(inputs["batch"]).astype(np.int64)

    src = edge_index[0].astype(np.int64)
    dst = edge_index[1].astype(np.int64)
    N = x.shape[0]
    G = u.shape[0]

    cnt = np.maximum(np.bincount(batch, minlength=G).astype(np.float32), 1.0)[:, None]

    # segment-sum plumbing (static across steps)
    e_order = np.argsort(dst, kind="stable")
    dsort = dst[e_order]
    uniq_d, starts_d = np.unique(dsort, return_index=True)
    ub, starts_b = np.unique(batch, return_index=True)

    def segsum_edges(vals):
        s = np.add.reduceat(vals[e_order], starts_d, axis=0)
        out = np.zeros((N, vals.shape[1]), np.float32)
        out[uniq_d] = s
        return out

    def segsum_nodes(vals):
        s = np.add.reduceat(vals, starts_b, axis=0)
        out = np.zeros((G, vals.shape[1]), np.float32)
        out[ub] = s
        return out

    key = E_LOC
    if key not in _NC_CACHE:
        _NC_CACHE[key] = build_nc(E_LOC)
    nc = _NC_CACHE[key]

    I128 = np.eye(H, dtype=np.float32)
    wts_np = np.ascontiguousarray(
        np.concatenate(
            [We[256:384], I128,
             eWih[:, 0:H], eWih[:, H:2 * H], eWih[:, 2 * H:3 * H],
             eWhh[:, 0:H], eWhh[:, H:2 * H], eWhh[:, 2 * H:3 * H]],
            axis=1,
        )
    ).astype(bfloat16)
    bias_np = np.ascontiguousarray(
        np.stack(
            [be,
             ebih[0:H] + ebhh[0:H],
             ebih[H:2 * H] + ebhh[H:2 * H],
             ebhh[2 * H:3 * H],
             ebih[2 * H:3 * H]],
            axis=1,
        )
    ).astype(np.float32)
    brow_np = np.ascontiguousarray(
        np.concatenate([be, ebhh[2 * H:3 * H], ebih[0:H] + ebhh[0:H], ebih[H:2 * H] + ebhh[H:2 * H], ebih[2 * H:3 * H]])[None, :]
    ).astype(bfloat16)

    ea_state = ea
    us = []
    for _step in range(3):
        U4 = u @ We[384:512]
        xa = x @ We[0:128] + U4[batch]
        xb = x @ We[128:256]
        gfull = xa[src] + xb[dst]

        in_maps = []
        for k in range(NCORES):
            sl = slice(k * E_SHARD, (k + 1) * E_SHARD)
            gk = np.zeros((E_LOC, H), np.float32)
            gk[:E_SHARD] = gfull[sl]
            eak = np.zeros((E_LOC, H), np.float32)
            eak[:E_SHARD] = ea_state[sl]
            in_maps.append(
                dict(
                    g=np.ascontiguousarray(gk.T).astype(bfloat16),
                    ea=np.ascontiguousarray(eak.T).astype(bfloat16),
                    wts=wts_np,
                    bias=bias_np,
                    brow=brow_np,
                )
            )
        import time as _time
        run = _get_runner(nc)
        _t0 = _time.perf_counter()
        results = run(in_maps)
        LAST_EXEC_NS.append(int((_time.perf_counter() - _t0) * 1e9))
        class _R:  # keep downstream shape
            pass
        res = _R()
        res.results = results
        ea_state = np.concatenate(
            [np.asarray(res.results[k]["eao"]).astype(np.float32).T[:E_SHARD]
             for k in range(NCORES)],
            axis=0,
        )

        # node model (host, fp32)
        agg = segsum_edges(ea_state)
        n_in = np.concatenate([x, agg, u[batch]], axis=1)
        x_out = np.maximum(n_in @ Wn + bn, 0.0)
        x = _gru_np(x_out, x, nWih, nWhh, nbih, nbhh)

        # global model (host, fp32)
        x_mean = segsum_nodes(x) / cnt
        g_in = np.concatenate([x_mean, u], axis=1)
        u_out = np.maximum(g_in @ Wg + bg, 0.0)
        u = _gru_np(u_out, u, gWih, gWhh, gbih, gbhh)
        us.append(u.copy())

    return np.stack(us, axis=1).astype(np.float32)

